# revision 31
# baseline (speedup 1.0000x reference)
# Bass/Trainium2 kernel for MHConvAttention (B=16, C=128, H=W=64, NH=8, OUT=512)
# Data-parallel over batch: 8 cores x 2 samples each.
#
# Per-sample layout: channels (128) on SBUF partitions, flattened spatial (4096)
# on the free dim. The two depthwise convs (3x3 CPE, 5x5 rel-pos) run as fp8
# DoubleRow matmuls: two taps are packed per instruction (two diagonal weight
# planes + two shifted input slices via an overlapping strided access pattern),
# halving tensor-engine rows per tap versus bf16/f32r. The 3x3 conv's center
# tap (+residual) stays exact: scalar-engine per-channel scale, fused back with
# the fp8 taps via one affine_then_add. The content-lambda path uses a
# transposed QKV GEMM; ECA channel attention folds into the out projection.
import os
import numpy as np

B, C, H, W = 16, 128, 64, 64
NH, HD, WIN, OUT = 8, 16, 5, 512
N = H * W
NCORES = 8
SPC = B // NCORES          # samples per core
NC8 = N // 512             # 512-wide chunks per sample
NJ = N // 128              # 128-wide chunks (transposed GEMM)
SCALING = HD ** (-0.5)
FS = 16.0                  # fp8 weight upscale (compensated in-kernel)

# 3x3 conv: outer-tap pairs (tap k = 3*dy+dx, center k=4 handled exactly).
# Entries: (tap_a, tap_b). Pair-dim byte stride in srcp8 (66-wide rows) is
# off(b)-off(a), off(k) = (k//3)*66 + k%3. Stride 1 is illegal on HW; all
# pairs here give stride 2 or 132.
PAIRS3 = [(0, 2), (1, 7), (3, 5), (6, 8)]
# 5x5 conv: tap k = 5*dy+dx in vpad8 (68-wide rows), off(k) = (k//5)*68 + k%5.
# Vertical pairs (stride 68) for rows 0-3, horizontal stride-2 pairs for row 4;
# tap 22 appears twice -- its second weight slot is zeroed host-side.
PAIRS5 = [(0, 5), (1, 6), (2, 7), (3, 8), (4, 9),
          (10, 15), (11, 16), (12, 17), (13, 18), (14, 19),
          (20, 22), (21, 23), (22, 24)]

_CACHE = {}


def _build_nc():
    import concourse.bass as bass
    import concourse.tile as tile
    import concourse.mybir as mybir
    from concourse import bacc

    f32 = mybir.dt.float32
    f32r = mybir.dt.float32r
    bf16 = mybir.dt.bfloat16
    fp8 = mybir.dt.float8e4
    Alu = mybir.AluOpType
    Act = mybir.ActivationFunctionType
    DR = mybir.MatmulPerfMode.DoubleRow

    def r(ap):
        return ap.bitcast(f32r)

    def pair_mv(tile3d, y0, tap_a, tap_b, row_w):
        """Overlapping [C, 2, 8, 64] moving AP for a DoubleRow tap pair."""
        k = 5 if row_w == 68 else 3
        dya, dxa = tap_a // k, tap_a % k
        dyb, dxb = tap_b // k, tap_b % k
        delta = (dyb - dya) * row_w + (dxb - dxa)
        sl = tile3d[:, y0 + dya : y0 + dya + 8, dxa : dxa + 64]
        mv = sl.unsqueeze(1).broadcast_to([C, 2, 8, 64])
        mv.ap[1] = [delta, 2]
        return mv

    nc = bacc.Bacc(trn_type="TRN2", target_bir_lowering=False, debug=False)

    src_d = nc.dram_tensor("src", [SPC, C, H + 2, W + 2], bf16, kind="ExternalInput").ap()
    src8_d = nc.dram_tensor("src8", [SPC, C, H + 2, W + 2], fp8, kind="ExternalInput").ap()
    d3p_d = nc.dram_tensor("d3p", [C, len(PAIRS3), 2, C], fp8, kind="ExternalInput").ap()
    cen_d = nc.dram_tensor("cen", [C, 1], f32, kind="ExternalInput").ap()
    d5p_d = nc.dram_tensor("d5p", [C, len(PAIRS5), 2, C], fp8, kind="ExternalInput").ap()
    wq_d = nc.dram_tensor("wq", [C, C], bf16, kind="ExternalInput").ap()
    wv_d = nc.dram_tensor("wv", [C, C], bf16, kind="ExternalInput").ap()
    wkv_d = nc.dram_tensor("wkv", [C, 2 * C], bf16, kind="ExternalInput").ap()
    w1_d = nc.dram_tensor("w1", [C, OUT], bf16, kind="ExternalInput").ap()
    w2_d = nc.dram_tensor("w2", [C, OUT], bf16, kind="ExternalInput").ap()
    mask_d = nc.dram_tensor("mask", [C, C], f32, kind="ExternalInput").ap()
    trid_d = nc.dram_tensor("trid", [C, C], f32, kind="ExternalInput").ap()
    out_d = nc.dram_tensor("out", [SPC, OUT, H, W], f32, kind="ExternalOutput").ap()
    out_v = out_d.rearrange("s o h w -> s o (h w)")

    with tile.TileContext(nc) as tc, __import__("contextlib").ExitStack() as ctx:
        wpool = ctx.enter_context(tc.tile_pool(name="w", bufs=1))
        srcp_pool = ctx.enter_context(tc.tile_pool(name="srcp", bufs=2))
        src8_pool = ctx.enter_context(tc.tile_pool(name="src8", bufs=2))
        s_pool = ctx.enter_context(tc.tile_pool(name="s", bufs=8))
        q_pool = ctx.enter_context(tc.tile_pool(name="q", bufs=8))
        r1_pool = ctx.enter_context(tc.tile_pool(name="r1", bufs=8))
        vpad_pool = ctx.enter_context(tc.tile_pool(name="vpad", bufs=1))
        eT_pool = ctx.enter_context(tc.tile_pool(name="eT", bufs=1))
        vT_pool = ctx.enter_context(tc.tile_pool(name="vT", bufs=1))
        tmp_pool = ctx.enter_context(tc.tile_pool(name="tmp", bufs=2))
        cen_pool = ctx.enter_context(tc.tile_pool(name="cen", bufs=4))
        stage_pool = ctx.enter_context(tc.tile_pool(name="stage", bufs=4))
        small_pool = ctx.enter_context(tc.tile_pool(name="small", bufs=2))
        ps_pool = ctx.enter_context(tc.tile_pool(name="ps", bufs=4, space="PSUM"))
        psT_pool = ctx.enter_context(tc.tile_pool(name="psT", bufs=3, space="PSUM"))
        psCL_pool = ctx.enter_context(tc.tile_pool(name="psCL", bufs=1, space="PSUM"))

        # ---- src prefetch (both samples) + weights, in order of first use.
        # src DMAs go first so their descriptors lead the rings; sample 0 is
        # quartered and triggered from gpsimd (earliest-booting engine). The
        # f32 srcp tile has no borders (nothing reads them); the fp8 src8 copy
        # carries the conv padding: gpsimd memsets borders, vector casts the
        # interior row bands as the DMA lands.
        # src arrives pre-padded (f32) and pre-quantized (fp8) from the host,
        # so every DMA is contiguous full rows and no on-chip cast is needed.
        # DMA-completion semaphores retire in issue order: the fp8 bands that
        # gate conv3 go first, then the f32 src, then the qkv weights, then
        # sample 1, then everything needed later.
        d3p_sb = wpool.tile([C, len(PAIRS3), 2, C], fp8)
        nc.sync.dma_start(d3p_sb[:], d3p_d[:])
        cen_sb = wpool.tile([C, 1], f32)
        nc.sync.dma_start(r(cen_sb[:]), r(cen_d[:]))
        # warm the scalar engine's Copy activation table off the critical path
        warm = wpool.tile([C, 1], f32)
        nc.scalar.memzero(warm[:])
        nc.scalar.activation(warm[:], warm[:], Act.Copy, bias=0.0)
        # pre-warm the PE clock gate (HAM) during the DMA wait: >=3us of
        # sustained matmul activity moves it to the full-speed state before
        # the first real matmul issues
        warmb = wpool.tile([C, 64], bf16)
        nc.gpsimd.memset(warmb[:], 0.0)
        ps_warm = psCL_pool.tile([64, 64], f32, tag="pscl")
        for _ in range(84):
            nc.tensor.matmul(ps_warm[:], warmb[:], warmb[:], start=True, stop=True)
        srcp_t = []
        src8_t = []
        hh = H // 2
        bands = [(0, 17), (17, 33), (33, 49), (49, 66)]
        src8_0 = src8_pool.tile([C, H + 2, W + 2], fp8, tag="src8")
        srcp_0 = srcp_pool.tile([C, H + 2, W + 2], bf16, tag="srcp")
        ra, rb = bands[0]
        nc.sync.dma_start(src8_0[:, ra:rb, :], src8_d[0, :, ra:rb])
        nc.sync.dma_start(srcp_0[:, ra:rb, :], src_d[0, :, ra:rb])
        wq_sb = wpool.tile([C, C], bf16)
        nc.sync.dma_start(wq_sb[:], wq_d[:])
        wv_sb = wpool.tile([C, C], bf16)
        nc.sync.dma_start(wv_sb[:], wv_d[:])
        wkv_sb = wpool.tile([C, 2 * C], bf16)
        nc.sync.dma_start(wkv_sb[:], wkv_d[:])
        for ra, rb in bands[1:]:
            nc.sync.dma_start(src8_0[:, ra:rb, :], src8_d[0, :, ra:rb])
            nc.sync.dma_start(srcp_0[:, ra:rb, :], src_d[0, :, ra:rb])
        src8_1 = src8_pool.tile([C, H + 2, W + 2], fp8, tag="src8")
        nc.sync.dma_start(src8_1[:, 0:33, :], src8_d[1, :, 0:33])
        nc.sync.dma_start(src8_1[:, 33:66, :], src8_d[1, :, 33:66])
        srcp_1 = srcp_pool.tile([C, H + 2, W + 2], bf16, tag="srcp")
        nc.sync.dma_start(srcp_1[:, 0:33, :], src_d[1, :, 0:33])
        nc.sync.dma_start(srcp_1[:, 33:66, :], src_d[1, :, 33:66])
        srcp_t = [srcp_0, srcp_1]
        src8_t = [src8_0, src8_1]
        mask_sb = wpool.tile([C, C], f32)
        nc.sync.dma_start(mask_sb[:], mask_d[:])
        trid_sb = wpool.tile([C, C], f32)
        nc.sync.dma_start(trid_sb[:], trid_d[:])
        d5p_sb = wpool.tile([C, len(PAIRS5), 2, C], fp8)
        nc.sync.dma_start(d5p_sb[:], d5p_d[:])
        w1_sb = wpool.tile([C, OUT], bf16)
        nc.sync.dma_start(w1_sb[:], w1_d[:])
        w2_sb = wpool.tile([C, OUT], bf16)
        nc.sync.dma_start(w2_sb[:], w2_d[:])

        for smp in range(SPC):
            srcp = srcp_t[smp]
            src8 = src8_t[smp]

            # ---- fused front phase, pipelined per 512-chunk:
            #   conv3(c8) -> [qkv + transposed-kv GEMMs of chunk c8-1]
            # so the tensor queue never head-of-line blocks on src DMA bands.
            # The center-tap activations also accumulate the spatial sum of
            # (1+w_c)*src per chunk into acc8 -- the ECA mean-pool for free
            # (the 1+w_c factor is divided back out inside trid host-side).
            acc8 = small_pool.tile([C, 8], f32, tag="acc8")
            vpad8 = vpad_pool.tile([C, H + 4, W + 4], fp8, tag="vpad")
            nc.vector.memset(vpad8[:, 0:2, :], 0.0)
            nc.vector.memset(vpad8[:, H + 2 : H + 4, :], 0.0)
            nc.vector.memset(vpad8[:, :, 0:2], 0.0)
            nc.vector.memset(vpad8[:, :, W + 2 : W + 4], 0.0)
            eT = eT_pool.tile([C, NJ, C], bf16, tag="eT")
            vT = vT_pool.tile([C, NJ, C + 1], bf16, tag="vT")
            nc.vector.memset(vT[:, :, C : C + 1], 1.0)
            s_t = []
            q_t = []

            def conv3_chunk(c8):
                y0 = 8 * c8
                ps = ps_pool.tile([C, 512], f32, tag="ps")
                for t, (ta, tb) in enumerate(PAIRS3):
                    nc.tensor.matmul(
                        ps[:],
                        d3p_sb[:, t, :, :],
                        pair_mv(src8, y0, ta, tb, W + 2),
                        start=(t == 0),
                        stop=(t == len(PAIRS3) - 1),
                        perf_mode=DR,
                    )
                cen_t = cen_pool.tile([C, 512], f32, tag="cen")
                nc.scalar.activation(
                    cen_t[:].rearrange("p (a b) -> p a b", a=8),
                    srcp[:, 1 + y0 : 1 + y0 + 8, 1 : W + 1],
                    Act.Copy, bias=0.0, scale=cen_sb[:],
                    accum_out=acc8[:, c8 : c8 + 1],
                )
                st = s_pool.tile([C, 512], bf16, tag="s")
                nc.vector.affine_then_add(st[:], ps[:], cen_t[:], 1.0 / FS, 0.0)
                s_t.append(st)

            def qkv_chunk(c8):
                psq = ps_pool.tile([C, 512], f32, tag="ps")
                nc.tensor.matmul(psq[:], wq_sb[:], s_t[c8][:], start=True, stop=True)
                qt = q_pool.tile([C, 512], bf16, tag="q")
                if c8 % 2 == 0:
                    nc.scalar.copy(qt[:], psq[:])
                else:
                    nc.vector.tensor_copy(qt[:], psq[:])
                q_t.append(qt)
                psv = ps_pool.tile([C, 512], f32, tag="ps")
                nc.tensor.matmul(psv[:], wv_sb[:], s_t[c8][:], start=True, stop=True)
                vdst = vpad8[:, 2 + 8 * c8 : 2 + 8 * c8 + 8, 2 : W + 2]
                vsrc = psv[:].rearrange("p (a b) -> p a b", a=8)
                if c8 % 2 == 0:
                    nc.vector.tensor_scalar(vdst, vsrc, 1.0 / FS, None, Alu.mult)
                else:
                    nc.scalar.activation(vdst, vsrc, Act.Copy, bias=0.0, scale=1.0 / FS)
                for j in range(4 * c8, 4 * c8 + 4, 2):
                    psT = psT_pool.tile([C, 2, 2 * C], f32, tag="psT")
                    for u in range(2):
                        lhs = s_t[c8][:, (j + u - 4 * c8) * 128 : (j + u - 4 * c8 + 1) * 128]
                        nc.tensor.matmul(
                            psT[:, u, :], lhs, wkv_sb[:], start=True, stop=True
                        )
                    nc.scalar.activation(eT[:, j : j + 2, :], psT[:, :, 0:C], Act.Exp)
                    nc.vector.tensor_copy(vT[:, j : j + 2, 0:C], psT[:, :, C : 2 * C])

            for c8 in range(NC8):
                conv3_chunk(c8)
                if c8 >= 1:
                    qkv_chunk(c8 - 1)
            qkv_chunk(NC8 - 1)

            # ---- ECA: ca = sigmoid(tridiag @ mean_pool(src)) ----
            # pool comes from the conv3 accum_out columns, summed on the
            # scalar engine (accum_out again) so the tensor queue never waits
            # on the busy vector queue; sigmoid is Exp (already the loaded
            # scalar table) + vector reciprocal to avoid a table swap.
            acc_s = small_pool.tile([C, 8], f32, tag="acc_s")
            pool_sum = small_pool.tile([C, 1], f32, tag="psum_vec")
            nc.scalar.activation(acc_s[:], acc8[:], Act.Copy, bias=0.0,
                                 accum_out=pool_sum[:])
            ps_eca = psCL_pool.tile([C, 1], f32, tag="pscl")
            nc.tensor.matmul(ps_eca[:], trid_sb[:], pool_sum[:], start=True, stop=True)
            emz = small_pool.tile([C, 1], f32, tag="emz")
            nc.scalar.activation(emz[:], ps_eca[:], Act.Exp, scale=-1.0)
            ca_d = small_pool.tile([C, 1], f32, tag="ca_d")
            nc.vector.tensor_scalar(ca_d[:], emz[:], 1.0, None, Alu.add)
            ca = small_pool.tile([C, 1], f32, tag="ca")
            nc.vector.reciprocal(ca[:], ca_d[:])
            w2p = stage_pool.tile([C, OUT], bf16, tag="w2p")
            nc.vector.tensor_scalar(w2p[:], w2_sb[:], ca[:], None, Alu.mult)

            # ---- first 5x5 conv group runs before the CL matmuls so the
            # tensor queue stays busy while eT/vT drains ----
            ps5_cache = {}

            # ---- content lambda: CL[i, o] (+ row sums in col 128) ----
            for hc in range(2):
                ps5h = ps_pool.tile([C, 512], f32, tag="ps")
                for t, (ta, tb) in enumerate(PAIRS5):
                    nc.tensor.matmul(
                        ps5h[:], d5p_sb[:, t, :, :], pair_mv(vpad8, 8 * hc, ta, tb, W + 4),
                        start=(t == 0), stop=(t == len(PAIRS5) - 1), perf_mode=DR,
                    )
                ps5_cache[hc] = ps5h
            ps_cl = psCL_pool.tile([C, C + 1], f32, tag="pscl")
            for j in range(NJ):
                nc.tensor.matmul(
                    ps_cl[:], eT[:, j, :], vT[:, j, :],
                    start=(j == 0), stop=(j == NJ - 1),
                )
            recip = small_pool.tile([C, 1], f32, tag="recip")
            nc.vector.reciprocal(recip[:], ps_cl[:, C : C + 1])
            cln_t = small_pool.tile([C, C], f32, tag="cln_t")
            nc.vector.tensor_scalar(cln_t[:], ps_cl[:, 0:C], recip[:], None, Alu.mult)
            cln = small_pool.tile([C, C], bf16, tag="cln")
            nc.vector.tensor_tensor(cln[:], cln_t[:], mask_sb[:], Alu.mult)

            # ---- 5x5 rel-pos conv (fp8 DoubleRow pairs) + content output,
            #      interleaved with the out projection in halves of 4 chunks.
            #      psc is issued one chunk late so the cln vector chain never
            #      blocks the tensor queue.
            def conv5_group(c8):
                if c8 in ps5_cache:
                    return ps5_cache.pop(c8)
                ps5 = ps_pool.tile([C, 512], f32, tag="ps")
                y0 = 8 * c8
                for t, (ta, tb) in enumerate(PAIRS5):
                    nc.tensor.matmul(
                        ps5[:],
                        d5p_sb[:, t, :, :],
                        pair_mv(vpad8, y0, ta, tb, W + 4),
                        start=(t == 0),
                        stop=(t == len(PAIRS5) - 1),
                        perf_mode=DR,
                    )
                return ps5

            def content_r1(c8, ps5):
                psc = psT_pool.tile([C, 512], f32, tag="psT")
                nc.tensor.matmul(psc[:], cln[:], q_t[c8][:], start=True, stop=True)
                tmp = tmp_pool.tile([C, 512], f32, tag="tmp")
                nc.vector.tensor_tensor(tmp[:], q_t[c8][:], ps5[:], Alu.mult)
                rt = r1_pool.tile([C, 512], bf16, tag="r1")
                nc.vector.tensor_tensor(rt[:], tmp[:], psc[:], Alu.add)
                return rt

            for half in range(2):
                chunks = list(range(half * 4, half * 4 + 4))
                r1_h = {}
                ps5_h = {}
                for c8 in chunks:
                    ps5_h[c8] = conv5_group(c8)
                for c8 in chunks:
                    r1_h[c8] = content_r1(c8, ps5_h[c8])

                dma_engs = [nc.sync, nc.scalar, nc.gpsimd]
                stgs = []
                for m in range(OUT // C):
                    stg = stage_pool.tile([C, 4 * 512], f32, tag="stage")
                    stgs.append(stg)
                for cc in range(4):
                    c8 = chunks[cc]
                    y0 = 8 * c8
                    for m in range(OUT // C):
                        if m % 2 == 0:
                            pso = ps_pool.tile([C, 512], f32, tag="ps")
                        else:
                            pso = psT_pool.tile([C, 512], f32, tag="psT")
                        nc.tensor.matmul(
                            pso[:], w1_sb[:, m * C : (m + 1) * C], r1_h[c8][:],
                            start=True, stop=False,
                        )
                        nc.tensor.matmul(
                            pso[:], w2p[:, m * C : (m + 1) * C],
                            srcp[:, 1 + y0 : 1 + y0 + 8, 1 : W + 1],
                            start=False, stop=True,
                        )
                        if (cc + m) % 2 == 0:
                            nc.scalar.copy(stgs[m][:, cc * 512 : (cc + 1) * 512], pso[:])
                        else:
                            nc.vector.tensor_copy(stgs[m][:, cc * 512 : (cc + 1) * 512], pso[:])
                        lo = half * 2048 + cc * 512
                        if cc == 3 and half == 1:
                            # final wave: split each store across both fast
                            # trigger engines so the tail drains in parallel
                            nc.sync.dma_start(
                                out_v[smp, m * C : (m + 1) * C, lo : lo + 256],
                                stgs[m][:, cc * 512 : cc * 512 + 256],
                            )
                            nc.scalar.dma_start(
                                out_v[smp, m * C : (m + 1) * C, lo + 256 : lo + 512],
                                stgs[m][:, cc * 512 + 256 : (cc + 1) * 512],
                            )
                        else:
                            eng = dma_engs[(m + cc) % (2 if cc == 3 else 3)]
                            eng.dma_start(
                                out_v[smp, m * C : (m + 1) * C, lo : lo + 512],
                                stgs[m][:, cc * 512 : (cc + 1) * 512],
                            )

    nc.compile()
    return nc


def _get_nc():
    if "nc" not in _CACHE:
        _CACHE["nc"] = _build_nc()
    return _CACHE["nc"]


def _host_weights(cpe_w, qkv_w, rel_pos, conv1d_w, out_w):
    import ml_dtypes

    cpe_w = np.asarray(cpe_w, np.float32)
    qkv_w = np.asarray(qkv_w, np.float32)
    rel_pos = np.asarray(rel_pos, np.float32)
    conv1d_w = np.asarray(conv1d_w, np.float32)
    out_w = np.asarray(out_w, np.float32)
    idx = np.arange(C)
    fp8 = ml_dtypes.float8_e4m3

    w3 = cpe_w[:, 0].reshape(C, 9)
    d3p = np.zeros([C, len(PAIRS3), 2, C], np.float32)
    for t, (ta, tb) in enumerate(PAIRS3):
        d3p[idx, t, 0, idx] = w3[:, ta] * FS
        d3p[idx, t, 1, idx] = w3[:, tb] * FS
    d3p = np.clip(d3p, -240, 240).astype(fp8)
    cen = np.ascontiguousarray((1.0 + w3[:, 4])[:, None])

    rp = rel_pos.reshape(HD, 25)[idx % HD]          # [C, 25]
    d5p = np.zeros([C, len(PAIRS5), 2, C], np.float32)
    seen = set()
    for t, (ta, tb) in enumerate(PAIRS5):
        if ta not in seen:
            d5p[idx, t, 0, idx] = rp[:, ta] * FS
            seen.add(ta)
        if tb not in seen:
            d5p[idx, t, 1, idx] = rp[:, tb] * FS
            seen.add(tb)
    assert seen == set(range(25))
    d5p = np.clip(d5p, -240, 240).astype(fp8)

    bf = ml_dtypes.bfloat16
    wq = np.ascontiguousarray(qkv_w[0:C, :].T.astype(bf))
    wv = np.ascontiguousarray(qkv_w[2 * C : 3 * C, :].T.astype(bf))
    wkv = np.ascontiguousarray(qkv_w[C : 3 * C, :].T.astype(bf))
    w1 = np.ascontiguousarray(out_w[:, 0:C].T.astype(bf))
    w2 = np.ascontiguousarray(out_w[:, C : 2 * C].T.astype(bf))

    mask = np.zeros([C, C], np.float32)
    for h in range(NH):
        mask[h * HD : (h + 1) * HD, h * HD : (h + 1) * HD] = SCALING

    trid = np.zeros([C, C], np.float32)
    trid[idx[:-1], idx[:-1] + 1] = conv1d_w[0]  # pool[c-1] contributes to ca[c]
    trid[idx, idx] = conv1d_w[1]
    trid[idx[1:], idx[1:] - 1] = conv1d_w[2]
    trid *= 1.0 / N
    # the kernel's pool-sum comes from the center-tap activation accumulator,
    # which carries an extra (1+w_center) factor per channel -- divide it out
    trid /= (1.0 + w3[:, 4])[None, :]
    return dict(d3p=d3p, cen=cen, d5p=d5p, wq=wq, wv=wv, wkv=wkv, w1=w1, w2=w2,
                mask=mask, trid=trid)


def kernel(src, cpe_w, qkv_w, rel_pos, conv1d_w, out_w):
    import ml_dtypes
    from concourse.bass_utils import run_bass_kernel_spmd

    src = np.asarray(src, np.float32)
    srcp_f = np.zeros([B, C, H + 2, W + 2], np.float32)
    srcp_f[:, :, 1 : H + 1, 1 : W + 1] = src
    src8 = np.clip(srcp_f, -240, 240).astype(ml_dtypes.float8_e4m3)
    srcp = srcp_f.astype(ml_dtypes.bfloat16)
    w = _host_weights(cpe_w, qkv_w, rel_pos, conv1d_w, out_w)
    nc = _get_nc()
    in_maps = [
        {
            "src": np.ascontiguousarray(srcp[i * SPC : (i + 1) * SPC]),
            "src8": np.ascontiguousarray(src8[i * SPC : (i + 1) * SPC]),
            **w,
        }
        for i in range(NCORES)
    ]
    trace = bool(os.environ.get("BASS_TRACE"))
    res = run_bass_kernel_spmd(nc, in_maps, list(range(NCORES)), trace=trace)
    _CACHE["last_result"] = res
    out = np.concatenate([res.results[i]["out"] for i in range(NCORES)], axis=0)
    return out


# revision 33
# speedup vs baseline: 1.0014x; 1.0014x over previous
# Bass/Trainium2 kernel for MHConvAttention (B=16, C=128, H=W=64, NH=8, OUT=512)
# Data-parallel over batch: 8 cores x 2 samples each.
#
# Per-sample layout: channels (128) on SBUF partitions, flattened spatial (4096)
# on the free dim. The two depthwise convs (3x3 CPE, 5x5 rel-pos) run as fp8
# DoubleRow matmuls: two taps are packed per instruction (two diagonal weight
# planes + two shifted input slices via an overlapping strided access pattern),
# halving tensor-engine rows per tap versus bf16/f32r. The 3x3 conv's center
# tap (+residual) stays exact: scalar-engine per-channel scale, fused back with
# the fp8 taps via one affine_then_add. The content-lambda path uses a
# transposed QKV GEMM; ECA channel attention folds into the out projection.
import os
import numpy as np

B, C, H, W = 16, 128, 64, 64
NH, HD, WIN, OUT = 8, 16, 5, 512
N = H * W
NCORES = 8
SPC = B // NCORES          # samples per core
NC8 = N // 512             # 512-wide chunks per sample
NJ = N // 128              # 128-wide chunks (transposed GEMM)
SCALING = HD ** (-0.5)
FS = 16.0                  # fp8 weight upscale (compensated in-kernel)

# 3x3 conv: outer-tap pairs (tap k = 3*dy+dx, center k=4 handled exactly).
# Entries: (tap_a, tap_b). Pair-dim byte stride in srcp8 (66-wide rows) is
# off(b)-off(a), off(k) = (k//3)*66 + k%3. Stride 1 is illegal on HW; all
# pairs here give stride 2 or 132.
PAIRS3 = [(0, 2), (1, 7), (3, 5), (6, 8)]
# 5x5 conv: tap k = 5*dy+dx in vpad8 (68-wide rows), off(k) = (k//5)*68 + k%5.
# Vertical pairs (stride 68) for rows 0-3, horizontal stride-2 pairs for row 4;
# tap 22 appears twice -- its second weight slot is zeroed host-side.
PAIRS5 = [(0, 5), (1, 6), (2, 7), (3, 8), (4, 9),
          (10, 15), (11, 16), (12, 17), (13, 18), (14, 19),
          (20, 22), (21, 23), (22, 24)]

_CACHE = {}


def _build_nc():
    import concourse.bass as bass
    import concourse.tile as tile
    import concourse.mybir as mybir
    from concourse import bacc

    f32 = mybir.dt.float32
    f32r = mybir.dt.float32r
    bf16 = mybir.dt.bfloat16
    fp8 = mybir.dt.float8e4
    Alu = mybir.AluOpType
    Act = mybir.ActivationFunctionType
    DR = mybir.MatmulPerfMode.DoubleRow

    def r(ap):
        return ap.bitcast(f32r)

    def pair_mv(tile3d, y0, tap_a, tap_b, row_w):
        """Overlapping [C, 2, 8, 64] moving AP for a DoubleRow tap pair."""
        k = 5 if row_w == 68 else 3
        dya, dxa = tap_a // k, tap_a % k
        dyb, dxb = tap_b // k, tap_b % k
        delta = (dyb - dya) * row_w + (dxb - dxa)
        sl = tile3d[:, y0 + dya : y0 + dya + 8, dxa : dxa + 64]
        mv = sl.unsqueeze(1).broadcast_to([C, 2, 8, 64])
        mv.ap[1] = [delta, 2]
        return mv

    nc = bacc.Bacc(trn_type="TRN2", target_bir_lowering=False, debug=False)

    src_d = nc.dram_tensor("src", [SPC, C, H + 2, W + 2], bf16, kind="ExternalInput").ap()
    src8_d = nc.dram_tensor("src8", [SPC, C, H + 2, W + 2], fp8, kind="ExternalInput").ap()
    d3p_d = nc.dram_tensor("d3p", [C, len(PAIRS3), 2, C], fp8, kind="ExternalInput").ap()
    cen_d = nc.dram_tensor("cen", [C, 1], f32, kind="ExternalInput").ap()
    d5p_d = nc.dram_tensor("d5p", [C, len(PAIRS5), 2, C], fp8, kind="ExternalInput").ap()
    wq_d = nc.dram_tensor("wq", [C, C], bf16, kind="ExternalInput").ap()
    wv_d = nc.dram_tensor("wv", [C, C], bf16, kind="ExternalInput").ap()
    wkv_d = nc.dram_tensor("wkv", [C, 2 * C], bf16, kind="ExternalInput").ap()
    w1_d = nc.dram_tensor("w1", [C, OUT], bf16, kind="ExternalInput").ap()
    w2_d = nc.dram_tensor("w2", [C, OUT], bf16, kind="ExternalInput").ap()
    mask_d = nc.dram_tensor("mask", [C, C], f32, kind="ExternalInput").ap()
    trid_d = nc.dram_tensor("trid", [C, C], f32, kind="ExternalInput").ap()
    out_d = nc.dram_tensor("out", [SPC, OUT, H, W], f32, kind="ExternalOutput").ap()
    out_v = out_d.rearrange("s o h w -> s o (h w)")

    with tile.TileContext(nc) as tc, __import__("contextlib").ExitStack() as ctx:
        wpool = ctx.enter_context(tc.tile_pool(name="w", bufs=1))
        srcp_pool = ctx.enter_context(tc.tile_pool(name="srcp", bufs=2))
        src8_pool = ctx.enter_context(tc.tile_pool(name="src8", bufs=2))
        s_pool = ctx.enter_context(tc.tile_pool(name="s", bufs=8))
        q_pool = ctx.enter_context(tc.tile_pool(name="q", bufs=8))
        r1_pool = ctx.enter_context(tc.tile_pool(name="r1", bufs=8))
        vpad_pool = ctx.enter_context(tc.tile_pool(name="vpad", bufs=1))
        eT_pool = ctx.enter_context(tc.tile_pool(name="eT", bufs=1))
        vT_pool = ctx.enter_context(tc.tile_pool(name="vT", bufs=1))
        tmp_pool = ctx.enter_context(tc.tile_pool(name="tmp", bufs=2))
        cen_pool = ctx.enter_context(tc.tile_pool(name="cen", bufs=4))
        stage_pool = ctx.enter_context(tc.tile_pool(name="stage", bufs=4))
        small_pool = ctx.enter_context(tc.tile_pool(name="small", bufs=2))
        ps_pool = ctx.enter_context(tc.tile_pool(name="ps", bufs=4, space="PSUM"))
        psT_pool = ctx.enter_context(tc.tile_pool(name="psT", bufs=3, space="PSUM"))
        psCL_pool = ctx.enter_context(tc.tile_pool(name="psCL", bufs=1, space="PSUM"))

        # ---- src prefetch (both samples) + weights, in order of first use.
        # src DMAs go first so their descriptors lead the rings; sample 0 is
        # quartered and triggered from gpsimd (earliest-booting engine). The
        # f32 srcp tile has no borders (nothing reads them); the fp8 src8 copy
        # carries the conv padding: gpsimd memsets borders, vector casts the
        # interior row bands as the DMA lands.
        # src arrives pre-padded (f32) and pre-quantized (fp8) from the host,
        # so every DMA is contiguous full rows and no on-chip cast is needed.
        # DMA-completion semaphores retire in issue order: the fp8 bands that
        # gate conv3 go first, then the f32 src, then the qkv weights, then
        # sample 1, then everything needed later.
        d3p_sb = wpool.tile([C, len(PAIRS3), 2, C], fp8)
        nc.sync.dma_start(d3p_sb[:], d3p_d[:])
        cen_sb = wpool.tile([C, 1], f32)
        nc.sync.dma_start(r(cen_sb[:]), r(cen_d[:]))
        # warm the scalar engine's Copy activation table off the critical path
        warm = wpool.tile([C, 1], f32)
        nc.scalar.memzero(warm[:])
        nc.scalar.activation(warm[:], warm[:], Act.Copy, bias=0.0)
        # pre-warm the PE clock gate (HAM) during the DMA wait: >=3us of
        # sustained matmul activity moves it to the full-speed state before
        # the first real matmul issues
        warmb = wpool.tile([C, 64], bf16)
        nc.gpsimd.memset(warmb[:], 0.0)
        ps_warm = psCL_pool.tile([64, 64], f32, tag="pscl")
        for _ in range(84):
            nc.tensor.matmul(ps_warm[:], warmb[:], warmb[:], start=True, stop=True)
        srcp_t = []
        src8_t = []
        hh = H // 2
        bands = [(0, 17), (17, 33), (33, 49), (49, 66)]
        src8_0 = src8_pool.tile([C, H + 2, W + 2], fp8, tag="src8")
        srcp_0 = srcp_pool.tile([C, H + 2, W + 2], bf16, tag="srcp")
        ra, rb = bands[0]
        nc.sync.dma_start(src8_0[:, ra:rb, :], src8_d[0, :, ra:rb])
        nc.sync.dma_start(srcp_0[:, ra:rb, :], src_d[0, :, ra:rb])
        wq_sb = wpool.tile([C, C], bf16)
        nc.sync.dma_start(wq_sb[:], wq_d[:])
        wv_sb = wpool.tile([C, C], bf16)
        nc.sync.dma_start(wv_sb[:], wv_d[:])
        wkv_sb = wpool.tile([C, 2 * C], bf16)
        nc.sync.dma_start(wkv_sb[:], wkv_d[:])
        for ra, rb in bands[1:]:
            nc.sync.dma_start(src8_0[:, ra:rb, :], src8_d[0, :, ra:rb])
            nc.sync.dma_start(srcp_0[:, ra:rb, :], src_d[0, :, ra:rb])
        src8_1 = src8_pool.tile([C, H + 2, W + 2], fp8, tag="src8")
        nc.sync.dma_start(src8_1[:, 0:33, :], src8_d[1, :, 0:33])
        nc.sync.dma_start(src8_1[:, 33:66, :], src8_d[1, :, 33:66])
        srcp_1 = srcp_pool.tile([C, H + 2, W + 2], bf16, tag="srcp")
        nc.sync.dma_start(srcp_1[:, 0:33, :], src_d[1, :, 0:33])
        nc.sync.dma_start(srcp_1[:, 33:66, :], src_d[1, :, 33:66])
        srcp_t = [srcp_0, srcp_1]
        src8_t = [src8_0, src8_1]
        mask_sb = wpool.tile([C, C], f32)
        nc.sync.dma_start(mask_sb[:], mask_d[:])
        trid_sb = wpool.tile([C, C], f32)
        nc.sync.dma_start(trid_sb[:], trid_d[:])
        d5p_sb = wpool.tile([C, len(PAIRS5), 2, C], fp8)
        nc.sync.dma_start(d5p_sb[:], d5p_d[:])
        w1_sb = wpool.tile([C, OUT], bf16)
        nc.sync.dma_start(w1_sb[:], w1_d[:])
        w2_sb = wpool.tile([C, OUT], bf16)
        nc.sync.dma_start(w2_sb[:], w2_d[:])

        for smp in range(SPC):
            srcp = srcp_t[smp]
            src8 = src8_t[smp]

            # ---- fused front phase, pipelined per 512-chunk:
            #   conv3(c8) -> [qkv + transposed-kv GEMMs of chunk c8-1]
            # so the tensor queue never head-of-line blocks on src DMA bands.
            # The center-tap activations also accumulate the spatial sum of
            # (1+w_c)*src per chunk into acc8 -- the ECA mean-pool for free
            # (the 1+w_c factor is divided back out inside trid host-side).
            acc8 = small_pool.tile([C, 8], f32, tag="acc8")
            vpad8 = vpad_pool.tile([C, H + 4, W + 4], fp8, tag="vpad")
            nc.vector.memset(vpad8[:, 0:2, :], 0.0)
            nc.vector.memset(vpad8[:, H + 2 : H + 4, :], 0.0)
            nc.vector.memset(vpad8[:, :, 0:2], 0.0)
            nc.vector.memset(vpad8[:, :, W + 2 : W + 4], 0.0)
            eT = eT_pool.tile([C, NJ, C], bf16, tag="eT")
            vT = vT_pool.tile([C, NJ, C + 1], bf16, tag="vT")
            nc.vector.memset(vT[:, :, C : C + 1], 1.0)
            s_t = []
            q_t = []

            def conv3_chunk(c8):
                y0 = 8 * c8
                ps = ps_pool.tile([C, 512], f32, tag="ps")
                for t, (ta, tb) in enumerate(PAIRS3):
                    nc.tensor.matmul(
                        ps[:],
                        d3p_sb[:, t, :, :],
                        pair_mv(src8, y0, ta, tb, W + 2),
                        start=(t == 0),
                        stop=(t == len(PAIRS3) - 1),
                        perf_mode=DR,
                    )
                cen_t = cen_pool.tile([C, 512], f32, tag="cen")
                nc.scalar.activation(
                    cen_t[:].rearrange("p (a b) -> p a b", a=8),
                    srcp[:, 1 + y0 : 1 + y0 + 8, 1 : W + 1],
                    Act.Copy, bias=0.0, scale=cen_sb[:],
                    accum_out=acc8[:, c8 : c8 + 1],
                )
                st = s_pool.tile([C, 512], bf16, tag="s")
                nc.vector.affine_then_add(st[:], ps[:], cen_t[:], 1.0 / FS, 0.0)
                s_t.append(st)

            def qkv_chunk(c8):
                psq = ps_pool.tile([C, 512], f32, tag="ps")
                nc.tensor.matmul(psq[:], wq_sb[:], s_t[c8][:], start=True, stop=True)
                qt = q_pool.tile([C, 512], bf16, tag="q")
                if c8 % 2 == 0:
                    nc.scalar.copy(qt[:], psq[:])
                else:
                    nc.vector.tensor_copy(qt[:], psq[:])
                q_t.append(qt)
                psv = ps_pool.tile([C, 512], f32, tag="ps")
                nc.tensor.matmul(psv[:], wv_sb[:], s_t[c8][:], start=True, stop=True)
                vdst = vpad8[:, 2 + 8 * c8 : 2 + 8 * c8 + 8, 2 : W + 2]
                vsrc = psv[:].rearrange("p (a b) -> p a b", a=8)
                if c8 % 2 == 0:
                    nc.vector.tensor_scalar(vdst, vsrc, 1.0 / FS, None, Alu.mult)
                else:
                    nc.scalar.activation(vdst, vsrc, Act.Copy, bias=0.0, scale=1.0 / FS)
                for j in range(4 * c8, 4 * c8 + 4, 2):
                    psT = psT_pool.tile([C, 2, 2 * C], f32, tag="psT")
                    for u in range(2):
                        lhs = s_t[c8][:, (j + u - 4 * c8) * 128 : (j + u - 4 * c8 + 1) * 128]
                        nc.tensor.matmul(
                            psT[:, u, :], lhs, wkv_sb[:], start=True, stop=True
                        )
                    nc.scalar.activation(eT[:, j : j + 2, :], psT[:, :, 0:C], Act.Exp)
                    nc.vector.tensor_copy(vT[:, j : j + 2, 0:C], psT[:, :, C : 2 * C])

            for c8 in range(NC8):
                conv3_chunk(c8)
                if c8 >= 1:
                    qkv_chunk(c8 - 1)
            qkv_chunk(NC8 - 1)

            # ---- ECA: ca = sigmoid(tridiag @ mean_pool(src)) ----
            # pool comes from the conv3 accum_out columns, summed on the
            # scalar engine (accum_out again) so the tensor queue never waits
            # on the busy vector queue; sigmoid is Exp (already the loaded
            # scalar table) + vector reciprocal to avoid a table swap.
            acc_s = small_pool.tile([C, 8], f32, tag="acc_s")
            pool_sum = small_pool.tile([C, 1], f32, tag="psum_vec")
            nc.scalar.activation(acc_s[:], acc8[:], Act.Copy, bias=0.0,
                                 accum_out=pool_sum[:])
            ps_eca = psCL_pool.tile([C, 1], f32, tag="pscl")
            nc.tensor.matmul(ps_eca[:], trid_sb[:], pool_sum[:], start=True, stop=True)
            emz = small_pool.tile([C, 1], f32, tag="emz")
            nc.scalar.activation(emz[:], ps_eca[:], Act.Exp, scale=-1.0)
            ca_d = small_pool.tile([C, 1], f32, tag="ca_d")
            nc.vector.tensor_scalar(ca_d[:], emz[:], 1.0, None, Alu.add)
            ca = small_pool.tile([C, 1], f32, tag="ca")
            nc.vector.reciprocal(ca[:], ca_d[:])
            w2p = stage_pool.tile([C, OUT], bf16, tag="w2p")
            nc.vector.tensor_scalar(w2p[:], w2_sb[:], ca[:], None, Alu.mult)

            # ---- first 5x5 conv group runs before the CL matmuls so the
            # tensor queue stays busy while eT/vT drains ----
            ps5_cache = {}

            # ---- content lambda: CL[i, o] (+ row sums in col 128) ----
            for hc in range(2):
                ps5h = ps_pool.tile([C, 512], f32, tag="ps")
                for t, (ta, tb) in enumerate(PAIRS5):
                    nc.tensor.matmul(
                        ps5h[:], d5p_sb[:, t, :, :], pair_mv(vpad8, 8 * hc, ta, tb, W + 4),
                        start=(t == 0), stop=(t == len(PAIRS5) - 1), perf_mode=DR,
                    )
                ps5_cache[hc] = ps5h
            ps_cl = psCL_pool.tile([C, C + 1], f32, tag="pscl")
            for j in range(NJ):
                nc.tensor.matmul(
                    ps_cl[:], eT[:, j, :], vT[:, j, :],
                    start=(j == 0), stop=(j == NJ - 1),
                )
            recip = small_pool.tile([C, 1], f32, tag="recip")
            nc.vector.reciprocal(recip[:], ps_cl[:, C : C + 1])
            cln_t = small_pool.tile([C, C], f32, tag="cln_t")
            nc.vector.tensor_scalar(cln_t[:], ps_cl[:, 0:C], recip[:], None, Alu.mult)
            cln = small_pool.tile([C, C], bf16, tag="cln")
            nc.vector.tensor_tensor(cln[:], cln_t[:], mask_sb[:], Alu.mult)

            # ---- 5x5 rel-pos conv (fp8 DoubleRow pairs) + content output,
            #      interleaved with the out projection in halves of 4 chunks.
            #      psc is issued one chunk late so the cln vector chain never
            #      blocks the tensor queue.
            def conv5_group(c8):
                if c8 in ps5_cache:
                    return ps5_cache.pop(c8)
                ps5 = ps_pool.tile([C, 512], f32, tag="ps")
                y0 = 8 * c8
                for t, (ta, tb) in enumerate(PAIRS5):
                    nc.tensor.matmul(
                        ps5[:],
                        d5p_sb[:, t, :, :],
                        pair_mv(vpad8, y0, ta, tb, W + 4),
                        start=(t == 0),
                        stop=(t == len(PAIRS5) - 1),
                        perf_mode=DR,
                    )
                return ps5

            def content_r1(c8, ps5):
                psc = psT_pool.tile([C, 512], f32, tag="psT")
                nc.tensor.matmul(psc[:], cln[:], q_t[c8][:], start=True, stop=True)
                tmp = tmp_pool.tile([C, 512], f32, tag="tmp")
                nc.vector.tensor_tensor(tmp[:], q_t[c8][:], ps5[:], Alu.mult)
                rt = r1_pool.tile([C, 512], bf16, tag="r1")
                nc.vector.tensor_tensor(rt[:], tmp[:], psc[:], Alu.add)
                return rt

            for half in range(2):
                chunks = list(range(half * 4, half * 4 + 4))
                r1_h = {}
                ps5_h = {}
                for c8 in chunks:
                    ps5_h[c8] = conv5_group(c8)
                for c8 in chunks:
                    r1_h[c8] = content_r1(c8, ps5_h[c8])

                dma_engs = [nc.sync, nc.scalar, nc.gpsimd]
                stgs = []
                for m in range(OUT // C):
                    stg = stage_pool.tile([C, 4 * 512], f32, tag="stage")
                    stgs.append(stg)
                for cc in range(4):
                    c8 = chunks[cc]
                    y0 = 8 * c8
                    for m in range(OUT // C):
                        if m % 2 == 0:
                            pso = ps_pool.tile([C, 512], f32, tag="ps")
                        else:
                            pso = psT_pool.tile([C, 512], f32, tag="psT")
                        nc.tensor.matmul(
                            pso[:], w1_sb[:, m * C : (m + 1) * C], r1_h[c8][:],
                            start=True, stop=False,
                        )
                        nc.tensor.matmul(
                            pso[:], w2p[:, m * C : (m + 1) * C],
                            srcp[:, 1 + y0 : 1 + y0 + 8, 1 : W + 1],
                            start=False, stop=True,
                        )
                        if (cc + m) % 2 == 0:
                            nc.scalar.copy(stgs[m][:, cc * 512 : (cc + 1) * 512], pso[:])
                        else:
                            nc.vector.tensor_copy(stgs[m][:, cc * 512 : (cc + 1) * 512], pso[:])
                        lo = half * 2048 + cc * 512
                        if cc == 3 and half == 1:
                            # final wave: split each store across both fast
                            # trigger engines so the tail drains in parallel
                            nc.sync.dma_start(
                                out_v[smp, m * C : (m + 1) * C, lo : lo + 256],
                                stgs[m][:, cc * 512 : cc * 512 + 256],
                            )
                            nc.scalar.dma_start(
                                out_v[smp, m * C : (m + 1) * C, lo + 256 : lo + 512],
                                stgs[m][:, cc * 512 + 256 : (cc + 1) * 512],
                            )
                        else:
                            eng = dma_engs[(m + cc) % (2 if cc == 3 else 3)]
                            eng.dma_start(
                                out_v[smp, m * C : (m + 1) * C, lo : lo + 512],
                                stgs[m][:, cc * 512 : (cc + 1) * 512],
                            )

    nc.compile()
    return nc


def _get_nc():
    if "nc" not in _CACHE:
        _CACHE["nc"] = _build_nc()
    return _CACHE["nc"]


def _host_weights(cpe_w, qkv_w, rel_pos, conv1d_w, out_w):
    import ml_dtypes

    cpe_w = np.asarray(cpe_w, np.float32)
    qkv_w = np.asarray(qkv_w, np.float32)
    rel_pos = np.asarray(rel_pos, np.float32)
    conv1d_w = np.asarray(conv1d_w, np.float32)
    out_w = np.asarray(out_w, np.float32)
    idx = np.arange(C)
    fp8 = ml_dtypes.float8_e4m3

    w3 = cpe_w[:, 0].reshape(C, 9)
    d3p = np.zeros([C, len(PAIRS3), 2, C], np.float32)
    for t, (ta, tb) in enumerate(PAIRS3):
        d3p[idx, t, 0, idx] = w3[:, ta] * FS
        d3p[idx, t, 1, idx] = w3[:, tb] * FS
    d3p = np.clip(d3p, -240, 240).astype(fp8)
    cen = np.ascontiguousarray((1.0 + w3[:, 4])[:, None])

    rp = rel_pos.reshape(HD, 25)[idx % HD]          # [C, 25]
    d5p = np.zeros([C, len(PAIRS5), 2, C], np.float32)
    seen = set()
    for t, (ta, tb) in enumerate(PAIRS5):
        if ta not in seen:
            d5p[idx, t, 0, idx] = rp[:, ta] * FS
            seen.add(ta)
        if tb not in seen:
            d5p[idx, t, 1, idx] = rp[:, tb] * FS
            seen.add(tb)
    assert seen == set(range(25))
    d5p = np.clip(d5p, -240, 240).astype(fp8)

    bf = ml_dtypes.bfloat16
    wq = np.ascontiguousarray(qkv_w[0:C, :].T.astype(bf))
    wv = np.ascontiguousarray(qkv_w[2 * C : 3 * C, :].T.astype(bf))
    wkv = np.ascontiguousarray(qkv_w[C : 3 * C, :].T.astype(bf))
    w1 = np.ascontiguousarray(out_w[:, 0:C].T.astype(bf))
    w2 = np.ascontiguousarray(out_w[:, C : 2 * C].T.astype(bf))

    mask = np.zeros([C, C], np.float32)
    for h in range(NH):
        mask[h * HD : (h + 1) * HD, h * HD : (h + 1) * HD] = SCALING

    trid = np.zeros([C, C], np.float32)
    trid[idx[:-1], idx[:-1] + 1] = conv1d_w[0]  # pool[c-1] contributes to ca[c]
    trid[idx, idx] = conv1d_w[1]
    trid[idx[1:], idx[1:] - 1] = conv1d_w[2]
    trid *= 1.0 / N
    # the kernel's pool-sum comes from the center-tap activation accumulator,
    # which carries an extra (1+w_center) factor per channel -- divide it out
    trid /= (1.0 + w3[:, 4])[None, :]
    return dict(d3p=d3p, cen=cen, d5p=d5p, wq=wq, wv=wv, wkv=wkv, w1=w1, w2=w2,
                mask=mask, trid=trid)


def kernel(src, cpe_w, qkv_w, rel_pos, conv1d_w, out_w):
    import ml_dtypes
    from concourse.bass_utils import run_bass_kernel_spmd

    src = np.asarray(src, np.float32)
    srcp_f = np.zeros([B, C, H + 2, W + 2], np.float32)
    srcp_f[:, :, 1 : H + 1, 1 : W + 1] = src
    src8 = np.clip(srcp_f, -240, 240).astype(ml_dtypes.float8_e4m3)
    srcp = srcp_f.astype(ml_dtypes.bfloat16)
    w = _host_weights(cpe_w, qkv_w, rel_pos, conv1d_w, out_w)
    nc = _get_nc()
    in_maps = [
        {
            "src": np.ascontiguousarray(srcp[i * SPC : (i + 1) * SPC]),
            "src8": np.ascontiguousarray(src8[i * SPC : (i + 1) * SPC]),
            **w,
        }
        for i in range(NCORES)
    ]
    trace = bool(os.environ.get("BASS_TRACE"))
    res = run_bass_kernel_spmd(nc, in_maps, list(range(NCORES)), trace=trace)
    _CACHE["last_result"] = res
    out = np.concatenate([res.results[i]["out"] for i in range(NCORES)], axis=0)
    return out
